# revision 11
# baseline (speedup 1.0000x reference)
"""Masked dot-product attention on 8 Trainium2 NeuronCores.

Problem shapes (hardcoded): queries/keys/values [128, 1024, 64] f32,
valid_lens [8] int (per-batch key valid length; BH = 8 batches x 16 heads).

Sharding: core c handles heads {b*16 + 2c, b*16 + 2c + 1} for all batches b
(16 heads/core, every batch present on every core -> uniform work, and one
compiled program serves all cores even with valid-len-dependent trip counts).

Host-side input prep (layout only; all attention math runs on device):
  - Q^T [BH, 64, 1024] with columns in "paired" order: column c*128+p holds
    query position (c//2)*256 + 2p + (c%2), so the output DMA writes >=512B
    contiguous runs; the permutation is undone by the output access pattern.
    K^T [BH, 64, 1024] natural order.
  - V is augmented with a ones column (softmax-denominator trick) and laid
    out partition-major: [BH, 128, 8, 65] so the whole-head V load is one
    contiguous >=2KB run per partition.
  - mask is an additive bias laid out as consumed: [128, b*8+c].

Per-head device pipeline (scores kept transposed, S^T[k, q]):
  per k-chunk c (only chunks below the batch's valid_len are computed):
    S^T[c] [128, 1024] = K^T_c.T @ Q^T            (PSUM, 2 matmuls, fp32r)
    P^T[c] = exp(S^T[c] * 1/8 + maskbias_c)       (ScalarE, bias = mask col)
  PV with ones-augmented V: out^T [65, q] += [V|1]_c.T @ P^T[c]; row 64
  accumulates sum(exp) = denominator. PE-transpose out^T back to [q, 65];
  reciprocal + scale on DVE -> [q, 64]; DMA out (descriptors un-permute q).

Schedule: the Activation engine (exp) is the bottleneck (~75us busy floor),
so the schedule minimizes ACT idle: head-0 input DMAs are issued before any
preamble work on the ACT/SP queues (split in halves so the first QK can
start early); heads are ordered big/small interleaved ending with the two
smallest; the final stretch interleaves the previous heads' PV groups
between the last heads' QK so exp never starves; the last head computes PV
in the q-partition orientation (P^T chunks as stationary weights, 4x/row
penalty but no transposes / no PSUM->SBUF copy) so the post-exp tail is
only reciprocal+multiply+store.

fp32r (TF32-like) matmul inputs: 4x faster than fp32, rel err ~2e-4.
Fully-masked batches (valid_len == 0) are patched on host.
"""

import numpy as np

P = 128          # partitions / k-chunk size
D = 64           # head dim
QL = 1024        # query length
KL = 1024        # key length
NB = 8           # batches
NH = 16          # heads per batch
NCORES = 8
HPC = 16         # heads per core
NCHUNK = KL // P # 8 k-chunks
NEG = -1.0e6

_POOLCFG = dict(io=3, pt=2, ot=4, fin=4, s=2, o=2, f=2)
_WARMUP = 4
_PVW_MAX_NCK = 3  # last-head PV-as-weights only when cheap enough


def _split_excess_waits(nc, max_waits=1):
    """This walrus (gen3) accepts only one sync-wait per instruction, but Tile
    emits up to 2 on compute ops and 5+ on the kernel-tail drain. Hoist excess
    on_wait entries onto fresh InstEventSemaphore ops on the same engine,
    inserted immediately before the offending instruction (same semantics:
    the engine stalls on each wait sequentially)."""
    import bass_rust
    import concourse.mybir as mybir

    n_split = 0
    for func in nc.m.functions:
        for block in func.blocks:
            out = []
            changed = False
            for inst in block.instructions:
                si = getattr(inst, "sync_info", None)
                waits = list(si.on_wait) if si is not None else []
                if len(waits) > max_waits:
                    changed = True
                    for w in waits[:-max_waits]:
                        n_split += 1
                        out.append(
                            mybir.InstEventSemaphore(
                                name=f"waitsplit_{n_split}_{inst.name}",
                                engine=inst.engine,
                                ins=[],
                                outs=[],
                                sync_info=bass_rust.SyncInfo(
                                    on_wait=[w], on_update=[]
                                ),
                            )
                        )
                    inst.sync_info = bass_rust.SyncInfo(
                        on_wait=waits[-max_waits:], on_update=list(si.on_update)
                    )
                out.append(inst)
            if changed:
                block.instructions = out
    return n_split


def _build(nc_chunks=None, reps=1):
    """Build the Bass program. nc_chunks: per-batch count of 128-wide k-chunks
    to compute (valid-len truncation)."""
    import concourse.bass as bass
    import concourse.mybir as mybir
    from concourse.tile import TileContext
    from concourse.masks import make_identity

    if nc_chunks is None:
        nc_chunks = [NCHUNK] * NB

    f32 = mybir.dt.float32
    f32r = mybir.dt.float32r
    Exp = mybir.ActivationFunctionType.Exp

    nc = bass.Bass(trn_type="TRN2")
    qd = nc.dram_tensor("qt", [HPC, D, QL], f32r, kind="ExternalInput")
    kd = nc.dram_tensor("kt", [HPC, D, KL], f32r, kind="ExternalInput")
    vd = nc.dram_tensor("v", [HPC, P, NCHUNK, D + 1], f32r, kind="ExternalInput")
    md = nc.dram_tensor("mask", [P, NB * NCHUNK], f32, kind="ExternalInput")
    od = nc.dram_tensor("out", [HPC, QL, D], f32, kind="ExternalOutput")

    cfg = dict(_POOLCFG)
    with TileContext(nc) as tc:
        with (
            tc.tile_pool(name="consts", bufs=1) as consts,
            tc.tile_pool(name="io", bufs=cfg["io"]) as io,
            tc.tile_pool(name="pt", bufs=cfg["pt"]) as ptp,
            tc.tile_pool(name="ot", bufs=cfg["ot"]) as otp,
            tc.tile_pool(name="fin", bufs=cfg["fin"]) as finp,
            tc.tile_pool(name="rc", bufs=4) as rcp,
            tc.tile_pool(name="ps_s", bufs=cfg["s"], space="PSUM") as ps_s,
            tc.tile_pool(name="ps_o", bufs=cfg["o"], space="PSUM") as ps_o,
            tc.tile_pool(name="ps_f", bufs=cfg["f"], space="PSUM") as ps_f,
        ):
            # Head order: interleave big and small (a head's finalize hides
            # under the NEXT head's exp phase), but end with the two smallest
            # heads so the un-hidden tail after the last exp is minimal.
            by_size = sorted(range(HPC), key=lambda h: -nc_chunks[h // 2])
            big, small = by_size[: HPC // 2], by_size[HPC // 2 :]
            order = [h for pair in zip(big[:-2], small[:-2]) for h in pair]
            order += [big[-2], big[-1], small[-2], small[-1]]

            # mask load first on the GPSIMD (SWDGE) queue - parallel with the
            # HWDGE input DMAs below.
            mask_sb = consts.tile([P, NB, NCHUNK], f32)
            nc.gpsimd.dma_start(
                out=mask_sb, in_=md.rearrange("p (b c) -> p b c", b=NB)
            )

            def emit_front(h, first=False):
                b = h // 2
                nck = nc_chunks[b]
                kt = io.tile([D, KL], f32r, tag="kt")
                qt = io.tile([D, QL], f32r, tag="qt")
                v1_sb = io.tile([P, NCHUNK, D + 1], f32r, tag="v")
                if first:
                    # The HWDGE ring-write stage is a single shared serial
                    # resource, so only the WRITE ORDER matters: qt half 0
                    # first (biggest item on the first-QK critical path),
                    # then kt chunk 0, then the rest.
                    nc.sync.dma_start(out=qt[:, 0:512], in_=qd[h][:, 0:512])
                    nc.sync.dma_start(out=kt[:, 0:P], in_=kd[h][:, 0:P])
                    nc.sync.dma_start(out=qt[:, 512:QL], in_=qd[h][:, 512:QL])
                    if nck > 1:
                        nc.sync.dma_start(
                            out=kt[:, P : nck * P], in_=kd[h][:, P : nck * P]
                        )
                else:
                    nc.sync.dma_start(
                        out=kt[:, 0 : nck * P], in_=kd[h][:, 0 : nck * P]
                    )
                    nc.sync.dma_start(out=qt, in_=qd[h])
                nc.sync.dma_start(
                    out=v1_sb[:, 0:nck, :], in_=vd[h][:, 0:nck, :]
                )
                return qt, kt, v1_sb

            # head-0 inputs BEFORE identity/priming so nothing delays them
            h0 = order[0]
            st0 = emit_front(h0, first=True)

            identity = consts.tile([P, P], f32)
            make_identity(nc, identity)
            # prime the ScalarE exp table load so it overlaps the input DMAs
            scratch = consts.tile([1, 1], f32)
            nc.vector.memset(scratch, 0.0)
            nc.scalar.activation(scratch, scratch, Exp)
            # prime the PE clock ramp with dummy matmuls sized to end as the
            # first real operands land
            warm = ps_f.tile([1, D], f32, tag="pf")
            for _ in range(_WARMUP):
                nc.tensor.matmul(
                    warm, identity[:, 0:1], identity[:, 0:D],
                    start=True, stop=True,
                )

            def emit_chunks(h, state, last=False, split_c0=False):
                b = h // 2
                nck = nc_chunks[b]
                qt, kt, v1_sb = state
                if last:
                    pt0 = ptp.tile([P, NCHUNK, 512], f32r, tag="pt0")
                    pt1 = ptp.tile([P, NCHUNK, 512], f32r, tag="pt1")
                    pts = [pt0, pt1]
                else:
                    pt = ptp.tile([P, NCHUNK, QL], f32r, tag="pt")
                    pts = [pt[:, :, 0:512], pt[:, :, 512:QL]]
                for c in range(nck):
                    ps = ps_s.tile([P, QL], f32, tag="s")
                    if last or (split_c0 and c == 0):
                        # per-half exp, interleaved with its own matmul so
                        # the half-exp's wait resolves against the right
                        # matmul (downstream unblocks earlier at the tail;
                        # the first exp doesn't wait for qt half 1)
                        for qh in range(2):
                            nc.tensor.matmul(
                                ps[:, qh * 512 : (qh + 1) * 512],
                                kt[:, c * P : (c + 1) * P],
                                qt[:, qh * 512 : (qh + 1) * 512],
                                start=True, stop=True,
                            )
                            nc.scalar.activation(
                                pts[qh][:, c, :],
                                ps[:, qh * 512 : (qh + 1) * 512],
                                Exp, bias=mask_sb[:, b, c : c + 1],
                                scale=0.125,
                            )
                    else:
                        nc.tensor.matmul(
                            ps[:, 0:512],
                            kt[:, c * P : (c + 1) * P],
                            qt[:, 0:512],
                            start=True, stop=True,
                        )
                        nc.tensor.matmul(
                            ps[:, 512:1024],
                            kt[:, c * P : (c + 1) * P],
                            qt[:, 512:1024],
                            start=True, stop=True,
                        )
                        nc.scalar.activation(
                            pt[:, c, :], ps,
                            Exp, bias=mask_sb[:, b, c : c + 1], scale=0.125,
                        )
                return pts, v1_sb

            def emit_pv(h, pt_v, qh, c_lo, c_hi, po):
                b = h // 2
                nck = nc_chunks[b]
                pts, v1_sb = pt_v
                for c in range(c_lo, c_hi):
                    nc.tensor.matmul(
                        po,
                        v1_sb[:, c, :],
                        pts[qh][:, c, :],
                        start=(c == 0), stop=(c == nck - 1),
                        skip_group_check=True,
                    )

            def emit_copy(po):
                oth = otp.tile([D + 1, 512], f32, tag="ot")
                nc.vector.tensor_copy(oth, po)
                return oth

            def emit_fin(h, ots, store_q=None, last=False):
                # transpose back, normalize, store (per q-half)
                store = store_q or nc.sync
                fin = finp.tile([P, 4, 2, D], f32, tag="fin")
                for qg in range(2):
                    pf = ps_f.tile([P, 4, D + 1], f32, tag="pf")
                    for j in range(4):
                        nc.tensor.transpose(
                            pf[:, j, :], ots[qg][:, j * P : (j + 1) * P],
                            identity[0 : D + 1, 0 : D + 1],
                        )
                    rc = rcp.tile([P, 4], f32, tag="rc")
                    nc.vector.reciprocal(rc, pf[:, :, D : D + 1])
                    nc.vector.tensor_mul(
                        fin[:, 2 * qg : 2 * qg + 2, :, :],
                        pf[:, :, 0:D],
                        rc[:, :, None].broadcast_to([P, 4, D]),
                    )
                    if last:
                        store.dma_start(
                            out=od[h].rearrange(
                                "(a p j) d -> p a j d", p=P, j=2
                            )[:, 2 * qg : 2 * qg + 2],
                            in_=fin[:, 2 * qg : 2 * qg + 2],
                        )
                if not last:
                    store.dma_start(
                        out=od[h].rearrange("(a p j) d -> p a j d", p=P, j=2),
                        in_=fin,
                    )

            def emit_pvfin(h, pt_v, store_q=None):
                ots = []
                for qh in range(2):
                    po = ps_o.tile([D + 1, 512], f32, tag="o")
                    emit_pv(h, pt_v, qh, 0, nc_chunks[h // 2], po)
                    ots.append(emit_copy(po))
                emit_fin(h, ots, store_q=store_q)

            def emit_pvw_mm(h, pt_v, qh):
                # Last-head path: PV with P^T chunks as stationary weights ->
                # out [128 q-part, 65] directly; normalize straight out of
                # PSUM; no transposes, no PSUM->SBUF copy. The q-column
                # pairing makes out partitions line up with the same store
                # pattern as the transposed path. Matmuls grouped (c-major
                # per a) so the chunk-c1 group resolves against its exp.
                b = h // 2
                nck = nc_chunks[b]
                pts, v1_sb = pt_v
                poq = ps_o.tile([P, 4, D + 1], f32, tag="o")
                for c in range(nck):
                    for a in range(4):
                        nc.tensor.matmul(
                            poq[:, a, :],
                            pts[qh][:, c, a * P : (a + 1) * P],
                            v1_sb[:, c, :],
                            start=(c == 0), stop=(c == nck - 1),
                            skip_group_check=True,
                        )
                return poq

            def emit_pvw_fin(h, poq, qh):
                rc = rcp.tile([P, 4], f32, tag="rc")
                nc.vector.reciprocal(rc, poq[:, :, D : D + 1])
                fin = finp.tile([P, 2, 2, D], f32, tag="fin")
                for a2 in range(2):
                    nc.vector.tensor_mul(
                        fin[:, a2, :, :],
                        poq[:, 2 * a2 : 2 * a2 + 2, 0:D],
                        rc[:, 2 * a2 : 2 * a2 + 2, None].broadcast_to(
                            [P, 2, D]
                        ),
                    )
                nc.sync.dma_start(
                    out=od[h].rearrange(
                        "(a p j) d -> p a j d", p=P, j=2
                    )[:, qh * 2 : qh * 2 + 2],
                    in_=fin,
                )

            # ---- steady-state loop ----
            pending_chunks = (h0, st0)
            pending_pv = None
            for h_rep in range(1, HPC - 1):
                h = order[h_rep]
                st = emit_front(h)
                ch, cst = pending_chunks
                out = emit_chunks(ch, cst, split_c0=(h_rep == 1))
                if pending_pv is not None:
                    emit_pvfin(*pending_pv)
                pending_pv = (ch, out)
                pending_chunks = (h, st)

            # ---- tail: A = order[-2], B = order[-1] (the two smallest) ----
            A, B = order[HPC - 2], order[HPC - 1]
            stA = pending_chunks[1]
            h5, pts5 = pending_pv
            nck5 = nc_chunks[h5 // 2]
            nckA = nc_chunks[A // 2]
            nckB = nc_chunks[B // 2]

            stB = emit_front(B)
            ptsA = emit_chunks(A, stA)
            # interleave h5's PV so QK(B) issues before the bulk of it
            po5_0 = ps_o.tile([D + 1, 512], f32, tag="o")
            emit_pv(h5, pts5, 0, 0, min(3, nck5), po5_0)
            use_pvw = nckB <= _PVW_MAX_NCK
            ptsB = emit_chunks(B, stB, last=use_pvw)
            emit_pv(h5, pts5, 0, min(3, nck5), nck5, po5_0)
            ot5_0 = emit_copy(po5_0)
            po5_1 = ps_o.tile([D + 1, 512], f32, tag="o")
            emit_pv(h5, pts5, 1, 0, nck5, po5_1)
            ot5_1 = emit_copy(po5_1)
            emit_fin(h5, [ot5_0, ot5_1])
            # A and B both finish via the PVW path when small: no PE
            # transposes and no PSUM->SBUF copies at the tail, so after the
            # final exp only one 4-matmul group + reciprocal/multiply/store
            # remain. A's stores complete during B's exp phase.
            if use_pvw and nckA <= _PVW_MAX_NCK:
                poqA0 = emit_pvw_mm(A, ptsA, 0)
                emit_pvw_fin(A, poqA0, 0)
                poqA1 = emit_pvw_mm(A, ptsA, 1)
                emit_pvw_fin(A, poqA1, 1)
                poqB0 = emit_pvw_mm(B, ptsB, 0)
                emit_pvw_fin(B, poqB0, 0)
                poqB1 = emit_pvw_mm(B, ptsB, 1)
                emit_pvw_fin(B, poqB1, 1)
            else:
                poA0 = ps_o.tile([D + 1, 512], f32, tag="o")
                emit_pv(A, ptsA, 0, 0, nckA, poA0)
                otA0 = emit_copy(poA0)
                poA1 = ps_o.tile([D + 1, 512], f32, tag="o")
                emit_pv(A, ptsA, 1, 0, nckA, poA1)
                otA1 = emit_copy(poA1)
                emit_fin(A, [otA0, otA1])
                if use_pvw:
                    poqB0 = emit_pvw_mm(B, ptsB, 0)
                    emit_pvw_fin(B, poqB0, 0)
                    poqB1 = emit_pvw_mm(B, ptsB, 1)
                    emit_pvw_fin(B, poqB1, 1)
                else:
                    emit_pvfin(B, ptsB)
    _split_excess_waits(nc)
    return nc


_CACHE = {}


def _get_nc(key, nc_chunks):
    if key not in _CACHE:
        _CACHE[key] = _build(nc_chunks)
    return _CACHE[key]


def _core_head_idx(c):
    return [b * NH + 2 * c + j for b in range(NB) for j in range(2)]


def _run(in_maps, nc, trace=False):
    from concourse.bass_utils import run_bass_kernel_spmd

    return run_bass_kernel_spmd(
        nc, in_maps, core_ids=list(range(NCORES)), trace=trace
    )


def _prepare(queries, keys, values, valid_lens):
    queries = np.asarray(queries, np.float32)
    keys = np.asarray(keys, np.float32)
    values = np.asarray(values, np.float32)
    vl = np.asarray(valid_lens).astype(np.int64)
    mask = np.where(
        np.arange(KL)[None, :] >= vl[:, None], np.float32(NEG), np.float32(0.0)
    ).astype(np.float32)
    # device layout [p, b*NCHUNK + c] = mask[b, c*128 + p]
    mask_dev = np.ascontiguousarray(
        mask.reshape(NB, NCHUNK, P).transpose(2, 0, 1).reshape(P, NB * NCHUNK)
    )
    nc_chunks = [max(1, int(min(NCHUNK, (int(v) + P - 1) // P))) for v in vl]
    bh = queries.shape[0]
    # Q^T with paired column order (see module docstring), K^T natural
    qtp = np.ascontiguousarray(
        queries.reshape(bh, 4, P, 2, D).transpose(0, 4, 1, 3, 2).reshape(
            bh, D, QL
        )
    )
    ktp = np.ascontiguousarray(keys.transpose(0, 2, 1))
    # V with ones column, partition-major: [BH, P, NCHUNK, D+1]
    v1 = np.concatenate(
        [values, np.ones((bh, KL, 1), np.float32)], axis=-1
    )
    v1p = np.ascontiguousarray(
        v1.reshape(bh, NCHUNK, P, D + 1).transpose(0, 2, 1, 3)
    )
    in_maps = []
    for c in range(NCORES):
        idx = _core_head_idx(c)
        in_maps.append(
            {
                "qt": qtp[idx],
                "kt": ktp[idx],
                "v": v1p[idx],
                "mask": mask_dev,
            }
        )
    return in_maps, nc_chunks, vl


def _gather(results, values, vl):
    out = np.empty((NB * NH, QL, D), np.float32)
    for c in range(NCORES):
        out[_core_head_idx(c)] = results[c]["out"]
    # fully-masked batches: reference softmax(-1e6 * ones) is uniform
    for b in range(NB):
        if vl[b] == 0:
            for hh in range(NH):
                bh = b * NH + hh
                out[bh] = np.asarray(values[bh], np.float32).mean(
                    axis=0, keepdims=True
                )
    return out


def kernel(queries, keys, values, valid_lens):
    in_maps, nc_chunks, vl = _prepare(queries, keys, values, valid_lens)
    nc = _get_nc(tuple(nc_chunks), nc_chunks)
    res = _run(in_maps, nc)
    return _gather(res.results, values, vl)


# revision 14
# speedup vs baseline: 1.0050x; 1.0050x over previous
"""Masked dot-product attention on 8 Trainium2 NeuronCores.

Problem shapes (hardcoded): queries/keys/values [128, 1024, 64] f32,
valid_lens [8] int (per-batch key valid length; BH = 8 batches x 16 heads).

Sharding: core c handles heads {b*16 + 2c, b*16 + 2c + 1} for all batches b
(16 heads/core, every batch present on every core -> uniform work, and one
compiled program serves all cores even with valid-len-dependent trip counts).

Host-side input prep (layout only; all attention math runs on device):
  - Q^T [BH, 64, 1024] with columns in "paired" order: column c*128+p holds
    query position (c//2)*256 + 2p + (c%2), so the output DMA writes >=512B
    contiguous runs; the permutation is undone by the output access pattern.
    K^T [BH, 64, 1024] natural order.
  - V is augmented with a ones column (softmax-denominator trick) and laid
    out partition-major: [BH, 128, 8, 65] so the whole-head V load is one
    contiguous >=2KB run per partition.
  - mask is an additive bias laid out as consumed: [128, b*8+c].

Per-head device pipeline (scores kept transposed, S^T[k, q]):
  per k-chunk c (only chunks below the batch's valid_len are computed):
    S^T[c] [128, 1024] = K^T_c.T @ Q^T            (PSUM, 2 matmuls, fp32r)
    P^T[c] = exp(S^T[c] * 1/8 + maskbias_c)       (ScalarE, bias = mask col)
  PV with ones-augmented V: out^T [65, q] += [V|1]_c.T @ P^T[c]; row 64
  accumulates sum(exp) = denominator. PE-transpose out^T back to [q, 65];
  reciprocal + scale on DVE -> [q, 64]; DMA out (descriptors un-permute q).

Schedule: the Activation engine (exp) is the bottleneck (~75us busy floor),
so the schedule minimizes ACT idle: head-0 input DMAs are issued before any
preamble work on the ACT/SP queues (split in halves so the first QK can
start early); heads are ordered big/small interleaved ending with the two
smallest; the final stretch interleaves the previous heads' PV groups
between the last heads' QK so exp never starves; the last head computes PV
in the q-partition orientation (P^T chunks as stationary weights, 4x/row
penalty but no transposes / no PSUM->SBUF copy) so the post-exp tail is
only reciprocal+multiply+store.

fp32r (TF32-like) matmul inputs: 4x faster than fp32, rel err ~2e-4.
Fully-masked batches (valid_len == 0) are patched on host.
"""

import numpy as np

P = 128          # partitions / k-chunk size
D = 64           # head dim
QL = 1024        # query length
KL = 1024        # key length
NB = 8           # batches
NH = 16          # heads per batch
NCORES = 8
HPC = 16         # heads per core
NCHUNK = KL // P # 8 k-chunks
NEG = -1.0e6

_POOLCFG = dict(io=3, pt=2, ot=4, fin=4, s=2, o=2, f=2)
_WARMUP = 4
_PVW_MAX_NCK = 3  # last-head PV-as-weights only when cheap enough


def _split_excess_waits(nc, max_waits=1):
    """This walrus (gen3) accepts only one sync-wait per instruction, but Tile
    emits up to 2 on compute ops and 5+ on the kernel-tail drain. Hoist excess
    on_wait entries onto fresh InstEventSemaphore ops on the same engine,
    inserted immediately before the offending instruction (same semantics:
    the engine stalls on each wait sequentially)."""
    import bass_rust
    import concourse.mybir as mybir

    n_split = 0
    for func in nc.m.functions:
        for block in func.blocks:
            out = []
            changed = False
            for inst in block.instructions:
                si = getattr(inst, "sync_info", None)
                waits = list(si.on_wait) if si is not None else []
                if len(waits) > max_waits:
                    changed = True
                    for w in waits[:-max_waits]:
                        n_split += 1
                        out.append(
                            mybir.InstEventSemaphore(
                                name=f"waitsplit_{n_split}_{inst.name}",
                                engine=inst.engine,
                                ins=[],
                                outs=[],
                                sync_info=bass_rust.SyncInfo(
                                    on_wait=[w], on_update=[]
                                ),
                            )
                        )
                    inst.sync_info = bass_rust.SyncInfo(
                        on_wait=waits[-max_waits:], on_update=list(si.on_update)
                    )
                out.append(inst)
            if changed:
                block.instructions = out
    return n_split


def _build(nc_chunks=None, reps=1):
    """Build the Bass program. nc_chunks: per-batch count of 128-wide k-chunks
    to compute (valid-len truncation)."""
    import concourse.bass as bass
    import concourse.mybir as mybir
    from concourse.tile import TileContext
    from concourse.masks import make_identity

    if nc_chunks is None:
        nc_chunks = [NCHUNK] * NB

    f32 = mybir.dt.float32
    f32r = mybir.dt.float32r
    Exp = mybir.ActivationFunctionType.Exp

    nc = bass.Bass(trn_type="TRN2")
    qd = nc.dram_tensor("qt", [HPC, D, QL], f32r, kind="ExternalInput")
    kd = nc.dram_tensor("kt", [HPC, D, KL], f32r, kind="ExternalInput")
    vd = nc.dram_tensor("v", [HPC, P, NCHUNK, D + 1], f32r, kind="ExternalInput")
    md = nc.dram_tensor("mask", [P, NB * NCHUNK], f32, kind="ExternalInput")
    od = nc.dram_tensor("out", [HPC, QL, D], f32, kind="ExternalOutput")

    cfg = dict(_POOLCFG)
    with TileContext(nc) as tc:
        with (
            tc.tile_pool(name="consts", bufs=1) as consts,
            tc.tile_pool(name="io", bufs=cfg["io"]) as io,
            tc.tile_pool(name="pt", bufs=cfg["pt"]) as ptp,
            tc.tile_pool(name="ot", bufs=cfg["ot"]) as otp,
            tc.tile_pool(name="fin", bufs=cfg["fin"]) as finp,
            tc.tile_pool(name="rc", bufs=4) as rcp,
            tc.tile_pool(name="ps_s", bufs=cfg["s"], space="PSUM") as ps_s,
            tc.tile_pool(name="ps_o", bufs=cfg["o"], space="PSUM") as ps_o,
            tc.tile_pool(name="ps_f", bufs=cfg["f"], space="PSUM") as ps_f,
        ):
            # Head order: interleave big and small (a head's finalize hides
            # under the NEXT head's exp phase), but end with the two smallest
            # heads so the un-hidden tail after the last exp is minimal.
            by_size = sorted(range(HPC), key=lambda h: -nc_chunks[h // 2])
            big, small = by_size[: HPC // 2], by_size[HPC // 2 :]
            order = [h for pair in zip(big[:-2], small[:-2]) for h in pair]
            order += [big[-2], big[-1], small[-2], small[-1]]

            # mask load first on the GPSIMD (SWDGE) queue - parallel with the
            # HWDGE input DMAs below.
            mask_sb = consts.tile([P, NB, NCHUNK], f32)
            nc.gpsimd.dma_start(
                out=mask_sb, in_=md.rearrange("p (b c) -> p b c", b=NB)
            )

            def emit_front(h, first=False):
                b = h // 2
                nck = nc_chunks[b]
                kt = io.tile([D, KL], f32r, tag="kt")
                qt = io.tile([D, QL], f32r, tag="qt")
                v1_sb = io.tile([P, NCHUNK, D + 1], f32r, tag="v")
                if first:
                    # The HWDGE ring-write stage is a single shared serial
                    # resource, so only the WRITE ORDER matters: qt half 0
                    # first (biggest item on the first-QK critical path),
                    # then kt chunk 0, then the rest.
                    nc.sync.dma_start(out=qt[:, 0:512], in_=qd[h][:, 0:512])
                    nc.sync.dma_start(out=kt[:, 0:P], in_=kd[h][:, 0:P])
                    nc.sync.dma_start(out=qt[:, 512:QL], in_=qd[h][:, 512:QL])
                    if nck > 1:
                        nc.sync.dma_start(
                            out=kt[:, P : nck * P], in_=kd[h][:, P : nck * P]
                        )
                else:
                    nc.sync.dma_start(
                        out=kt[:, 0 : nck * P], in_=kd[h][:, 0 : nck * P]
                    )
                    nc.sync.dma_start(out=qt, in_=qd[h])
                nc.sync.dma_start(
                    out=v1_sb[:, 0:nck, :], in_=vd[h][:, 0:nck, :]
                )
                return qt, kt, v1_sb

            # head-0 inputs BEFORE identity/priming so nothing delays them
            h0 = order[0]
            st0 = emit_front(h0, first=True)

            identity = consts.tile([P, P], f32)
            make_identity(nc, identity)
            # prime the ScalarE exp table load so it overlaps the input DMAs
            scratch = consts.tile([1, 1], f32)
            nc.vector.memset(scratch, 0.0)
            nc.scalar.activation(scratch, scratch, Exp)
            # prime the PE clock ramp with dummy matmuls sized to end as the
            # first real operands land
            warm = ps_f.tile([1, D], f32, tag="pf")
            for _ in range(_WARMUP):
                nc.tensor.matmul(
                    warm, identity[:, 0:1], identity[:, 0:D],
                    start=True, stop=True,
                )

            def emit_chunks(h, state, last=False, split_c0=False):
                b = h // 2
                nck = nc_chunks[b]
                qt, kt, v1_sb = state
                if last:
                    pt0 = ptp.tile([P, NCHUNK, 512], f32r, tag="pt0")
                    pt1 = ptp.tile([P, NCHUNK, 512], f32r, tag="pt1")
                    pts = [pt0, pt1]
                else:
                    pt = ptp.tile([P, NCHUNK, QL], f32r, tag="pt")
                    pts = [pt[:, :, 0:512], pt[:, :, 512:QL]]
                for c in range(nck):
                    ps = ps_s.tile([P, QL], f32, tag="s")
                    if last or (split_c0 and c == 0):
                        # per-half exp, interleaved with its own matmul so
                        # the half-exp's wait resolves against the right
                        # matmul (downstream unblocks earlier at the tail;
                        # the first exp doesn't wait for qt half 1)
                        for qh in range(2):
                            nc.tensor.matmul(
                                ps[:, qh * 512 : (qh + 1) * 512],
                                kt[:, c * P : (c + 1) * P],
                                qt[:, qh * 512 : (qh + 1) * 512],
                                start=True, stop=True,
                            )
                            nc.scalar.activation(
                                pts[qh][:, c, :],
                                ps[:, qh * 512 : (qh + 1) * 512],
                                Exp, bias=mask_sb[:, b, c : c + 1],
                                scale=0.125,
                            )
                    else:
                        nc.tensor.matmul(
                            ps[:, 0:512],
                            kt[:, c * P : (c + 1) * P],
                            qt[:, 0:512],
                            start=True, stop=True,
                        )
                        nc.tensor.matmul(
                            ps[:, 512:1024],
                            kt[:, c * P : (c + 1) * P],
                            qt[:, 512:1024],
                            start=True, stop=True,
                        )
                        nc.scalar.activation(
                            pt[:, c, :], ps,
                            Exp, bias=mask_sb[:, b, c : c + 1], scale=0.125,
                        )
                return pts, v1_sb

            def emit_pv(h, pt_v, qh, c_lo, c_hi, po):
                b = h // 2
                nck = nc_chunks[b]
                pts, v1_sb = pt_v
                for c in range(c_lo, c_hi):
                    nc.tensor.matmul(
                        po,
                        v1_sb[:, c, :],
                        pts[qh][:, c, :],
                        start=(c == 0), stop=(c == nck - 1),
                        skip_group_check=True,
                    )

            def emit_copy(po):
                oth = otp.tile([D + 1, 512], f32, tag="ot")
                nc.vector.tensor_copy(oth, po)
                return oth

            def emit_fin(h, ots, mul_eng=None):
                # transpose back, normalize, store (per q-half)
                eng = mul_eng or nc.vector
                fin = finp.tile([P, 4, 2, D], f32, tag="fin")
                for qg in range(2):
                    pf = ps_f.tile([P, 4, D + 1], f32, tag="pf")
                    for j in range(4):
                        nc.tensor.transpose(
                            pf[:, j, :], ots[qg][:, j * P : (j + 1) * P],
                            identity[0 : D + 1, 0 : D + 1],
                        )
                    rc = rcp.tile([P, 4], f32, tag="rc")
                    nc.vector.reciprocal(rc, pf[:, :, D : D + 1])
                    eng.tensor_mul(
                        fin[:, 2 * qg : 2 * qg + 2, :, :],
                        pf[:, :, 0:D],
                        rc[:, :, None].broadcast_to([P, 4, D]),
                    )
                nc.sync.dma_start(
                    out=od[h].rearrange("(a p j) d -> p a j d", p=P, j=2),
                    in_=fin,
                )

            def emit_pvfin(h, pt_v):
                ots = []
                for qh in range(2):
                    po = ps_o.tile([D + 1, 512], f32, tag="o")
                    emit_pv(h, pt_v, qh, 0, nc_chunks[h // 2], po)
                    ots.append(emit_copy(po))
                emit_fin(h, ots)

            def emit_pvw_mm(h, pt_v, qh):
                # Last-head path: PV with P^T chunks as stationary weights ->
                # out [128 q-part, 65] directly; normalize straight out of
                # PSUM; no transposes, no PSUM->SBUF copy. The q-column
                # pairing makes out partitions line up with the same store
                # pattern as the transposed path. Matmuls grouped (c-major
                # per a) so the chunk-c1 group resolves against its exp.
                b = h // 2
                nck = nc_chunks[b]
                pts, v1_sb = pt_v
                poq = ps_o.tile([P, 4, D + 1], f32, tag="o")
                for c in range(nck):
                    for a in range(4):
                        nc.tensor.matmul(
                            poq[:, a, :],
                            pts[qh][:, c, a * P : (a + 1) * P],
                            v1_sb[:, c, :],
                            start=(c == 0), stop=(c == nck - 1),
                            skip_group_check=True,
                        )
                return poq

            def emit_pvw_fin(h, poq, qh):
                rc = rcp.tile([P, 4], f32, tag="rc")
                nc.vector.reciprocal(rc, poq[:, :, D : D + 1])
                fin = finp.tile([P, 2, 2, D], f32, tag="fin")
                for a2 in range(2):
                    nc.vector.tensor_mul(
                        fin[:, a2, :, :],
                        poq[:, 2 * a2 : 2 * a2 + 2, 0:D],
                        rc[:, 2 * a2 : 2 * a2 + 2, None].broadcast_to(
                            [P, 2, D]
                        ),
                    )
                nc.sync.dma_start(
                    out=od[h].rearrange(
                        "(a p j) d -> p a j d", p=P, j=2
                    )[:, qh * 2 : qh * 2 + 2],
                    in_=fin,
                )

            # ---- steady-state loop ----
            pending_chunks = (h0, st0)
            pending_pv = None
            for h_rep in range(1, HPC - 1):
                h = order[h_rep]
                st = emit_front(h)
                ch, cst = pending_chunks
                out = emit_chunks(ch, cst, split_c0=(h_rep == 1))
                if pending_pv is not None:
                    emit_pvfin(*pending_pv)
                pending_pv = (ch, out)
                pending_chunks = (h, st)

            # ---- tail: A = order[-2], B = order[-1] (the two smallest) ----
            A, B = order[HPC - 2], order[HPC - 1]
            stA = pending_chunks[1]
            h5, pts5 = pending_pv
            nck5 = nc_chunks[h5 // 2]
            nckA = nc_chunks[A // 2]
            nckB = nc_chunks[B // 2]

            stB = emit_front(B)
            ptsA = emit_chunks(A, stA)
            # interleave h5's PV so QK(B) issues before the bulk of it
            po5_0 = ps_o.tile([D + 1, 512], f32, tag="o")
            emit_pv(h5, pts5, 0, 0, min(3, nck5), po5_0)
            use_pvw = nckB <= _PVW_MAX_NCK
            ptsB = emit_chunks(B, stB, last=use_pvw)
            emit_pv(h5, pts5, 0, min(3, nck5), nck5, po5_0)
            ot5_0 = emit_copy(po5_0)
            po5_1 = ps_o.tile([D + 1, 512], f32, tag="o")
            emit_pv(h5, pts5, 1, 0, nck5, po5_1)
            ot5_1 = emit_copy(po5_1)
            emit_fin(h5, [ot5_0, ot5_1])
            # A's PV + copies (hide under B's exps); A's normalize runs on
            # the Pool engine so the DVE only carries B's short tail chain.
            poA0 = ps_o.tile([D + 1, 512], f32, tag="o")
            emit_pv(A, ptsA, 0, 0, nckA, poA0)
            otA0 = emit_copy(poA0)
            poA1 = ps_o.tile([D + 1, 512], f32, tag="o")
            emit_pv(A, ptsA, 1, 0, nckA, poA1)
            otA1 = emit_copy(poA1)
            if use_pvw:
                # PE order: PVW(B,qh0), PVW(B,qh1 chunk<last), T(A),
                # PVW(B,qh1 last chunk) - the final exp-gated group is the
                # only PE work left after the last exp.
                poqB0 = emit_pvw_mm(B, ptsB, 0)
                emit_pvw_fin(B, poqB0, 0)
                poqB1 = ps_o.tile([P, 4, D + 1], f32, tag="o")
                vB = ptsB[1]
                for c in range(nckB - 1):
                    for a in range(4):
                        nc.tensor.matmul(
                            poqB1[:, a, :],
                            ptsB[0][1][:, c, a * P : (a + 1) * P],
                            vB[:, c, :],
                            start=(c == 0), stop=False,
                            skip_group_check=True,
                        )
                emit_fin(A, [otA0, otA1], mul_eng=nc.gpsimd)
                cl = nckB - 1
                for a in range(4):
                    nc.tensor.matmul(
                        poqB1[:, a, :],
                        ptsB[0][1][:, cl, a * P : (a + 1) * P],
                        vB[:, cl, :],
                        start=(cl == 0), stop=True,
                        skip_group_check=True,
                    )
                emit_pvw_fin(B, poqB1, 1)
            else:
                emit_fin(A, [otA0, otA1])
                emit_pvfin(B, ptsB)
    _split_excess_waits(nc)
    return nc


_CACHE = {}


def _get_nc(key, nc_chunks):
    if key not in _CACHE:
        _CACHE[key] = _build(nc_chunks)
    return _CACHE[key]


def _core_head_idx(c):
    return [b * NH + 2 * c + j for b in range(NB) for j in range(2)]


def _run(in_maps, nc, trace=False):
    from concourse.bass_utils import run_bass_kernel_spmd

    return run_bass_kernel_spmd(
        nc, in_maps, core_ids=list(range(NCORES)), trace=trace
    )


def _prepare(queries, keys, values, valid_lens):
    queries = np.asarray(queries, np.float32)
    keys = np.asarray(keys, np.float32)
    values = np.asarray(values, np.float32)
    vl = np.asarray(valid_lens).astype(np.int64)
    mask = np.where(
        np.arange(KL)[None, :] >= vl[:, None], np.float32(NEG), np.float32(0.0)
    ).astype(np.float32)
    # device layout [p, b*NCHUNK + c] = mask[b, c*128 + p]
    mask_dev = np.ascontiguousarray(
        mask.reshape(NB, NCHUNK, P).transpose(2, 0, 1).reshape(P, NB * NCHUNK)
    )
    nc_chunks = [max(1, int(min(NCHUNK, (int(v) + P - 1) // P))) for v in vl]
    bh = queries.shape[0]
    # Q^T with paired column order (see module docstring), K^T natural
    qtp = np.ascontiguousarray(
        queries.reshape(bh, 4, P, 2, D).transpose(0, 4, 1, 3, 2).reshape(
            bh, D, QL
        )
    )
    ktp = np.ascontiguousarray(keys.transpose(0, 2, 1))
    # V with ones column, partition-major: [BH, P, NCHUNK, D+1]
    v1 = np.concatenate(
        [values, np.ones((bh, KL, 1), np.float32)], axis=-1
    )
    v1p = np.ascontiguousarray(
        v1.reshape(bh, NCHUNK, P, D + 1).transpose(0, 2, 1, 3)
    )
    in_maps = []
    for c in range(NCORES):
        idx = _core_head_idx(c)
        in_maps.append(
            {
                "qt": qtp[idx],
                "kt": ktp[idx],
                "v": v1p[idx],
                "mask": mask_dev,
            }
        )
    return in_maps, nc_chunks, vl


def _gather(results, values, vl):
    out = np.empty((NB * NH, QL, D), np.float32)
    for c in range(NCORES):
        out[_core_head_idx(c)] = results[c]["out"]
    # fully-masked batches: reference softmax(-1e6 * ones) is uniform
    for b in range(NB):
        if vl[b] == 0:
            for hh in range(NH):
                bh = b * NH + hh
                out[bh] = np.asarray(values[bh], np.float32).mean(
                    axis=0, keepdims=True
                )
    return out


def kernel(queries, keys, values, valid_lens):
    in_maps, nc_chunks, vl = _prepare(queries, keys, values, valid_lens)
    nc = _get_nc(tuple(nc_chunks), nc_chunks)
    res = _run(in_maps, nc)
    return _gather(res.results, values, vl)
